# revision 33
# baseline (speedup 1.0000x reference)
# Trainium2 Bass kernel for nn_DeformablePatchEmbed_GELU (deformable patch
# embed + BatchNorm(batch stats) + exact GELU), data-parallel over 8 cores.
#
# Algorithm (device side, per core, B_loc=8 images):
#   For each output row ho (14 chunks of 112 positions = 14 wo x 8 b):
#     - DMA the padded 20x20x3 pixel window of every position into SBUF
#       [112 part, 1200 free] (host pre-pads x by 2 -> OOB reads are zeros).
#     - PE-transpose the interior 16x16x3 patch (flat order (c,ki,kj)) to
#       patchT tiles [128, 112] and matmul (fp32r) with the offset-conv
#       weight [768,512] -> offsets [112, 512] = [dy(256) | dx(256)].
#     - Bilinear sampling is decomposed over integer taps s in [-2,2]^2 with
#       hat weights: sampled = sum_s Hat(dy-sy)*Hat(dx-sx) * x[base+s],
#       Hat(u) = relu(1-|u|).  Hats built on ScalarE (Abs,Relu), the 25
#       masked MACs stream on VectorE against strided views of the window.
#     - PE-transpose sampled [112,768] -> matmul (fp32r) with dconv weight
#       (flat order (ki,kj,c)) -> y [112, 768]; stash in SBUF.
#     - BN partial sums: ones-vector matmuls give sum(y), sum(y^2) per o.
#   AllReduce (8 cores) of the 1536 partial sums -> global BN stats ->
#   per-o scale/shift; GELU on ScalarE (exact-erf LUT); DMA out [pos, o].
import numpy as np

import concourse.bacc as bacc
import concourse.bass as bass
import concourse.tile as tile
from concourse import mybir
from concourse.bass_utils import run_bass_kernel_spmd

F32 = mybir.dt.float32
F32R = mybir.dt.float32r
AF = mybir.ActivationFunctionType

# problem dims (hardcoded per contract)
B, C, H, W = 64, 3, 224, 224
O = 768
PATCH = 16
NCORES = 8
BL = B // NCORES            # 8 images per core
HO = WO = 14
PCH = WO * BL               # 112 positions per chunk (wo major, b minor)
NCHUNK = HO                 # 14
PAD = 2
HP = H + 2 * PAD            # 228
J = 768                     # patch flat size
NTOT = float(B * HO * WO)   # 12544 positions globally (BN denominator)
EPS = 1e-5
WIN = 20                    # window side
WROW = WIN * C              # 60
NWIN = WIN * WIN * C        # 1200

_CACHE = {}


def _mkap(handle_ap, offset, dims):
    return bass.AP(tensor=handle_ap.tensor, offset=offset, ap=[list(d) for d in dims])


def _build(n_cores=NCORES):
    nc = bacc.Bacc("TRN2", target_bir_lowering=False, debug=False, num_devices=n_cores)
    xwin = nc.dram_tensor("xwin", [NCHUNK, PCH, NWIN], F32, kind="ExternalInput")
    woff = nc.dram_tensor("woff", [J, 512], F32, kind="ExternalInput")
    wdm = nc.dram_tensor("wdm", [J, O], F32, kind="ExternalInput")
    offb = nc.dram_tensor("offb", [512], F32, kind="ExternalInput")
    bng = nc.dram_tensor("bng", [O], F32, kind="ExternalInput")
    bnb = nc.dram_tensor("bnb", [O], F32, kind="ExternalInput")
    ident = nc.dram_tensor("ident", [PCH, PCH], F32, kind="ExternalInput")
    outd = nc.dram_tensor("out", [BL, HO * WO, O], F32, kind="ExternalOutput")

    outd_b = outd[:]

    from contextlib import ExitStack
    with tile.TileContext(nc) as tc:
        with ExitStack() as ctx:
            consts = ctx.enter_context(tc.tile_pool(name="consts", bufs=1))
            wpool = ctx.enter_context(tc.tile_pool(name="wpool", bufs=2))
            ptpool = ctx.enter_context(tc.tile_pool(name="ptpool", bufs=2))
            dpool = ctx.enter_context(tc.tile_pool(name="dpool", bufs=2))
            hpool = ctx.enter_context(tc.tile_pool(name="hpool", bufs=2))
            mpool = ctx.enter_context(tc.tile_pool(name="mpool", bufs=3))
            tpool = ctx.enter_context(tc.tile_pool(name="tpool", bufs=2))
            apool = ctx.enter_context(tc.tile_pool(name="apool", bufs=2))
            stpool = ctx.enter_context(tc.tile_pool(name="stpool", bufs=2))
            ypool = ctx.enter_context(tc.tile_pool(name="ypool", bufs=NCHUNK))
            sqpool = ctx.enter_context(tc.tile_pool(name="sqpool", bufs=1))
            cpool = ctx.enter_context(tc.tile_pool(name="cpool", bufs=2))
            fpool = ctx.enter_context(tc.tile_pool(name="fpool", bufs=1))
            ps_t = ctx.enter_context(tc.tile_pool(name="ps_t", bufs=2, space="PSUM"))
            ps_off = ctx.enter_context(tc.tile_pool(name="ps_off", bufs=1, space="PSUM"))
            ps_y = ctx.enter_context(tc.tile_pool(name="ps_y", bufs=2, space="PSUM"))
            ps_s = ctx.enter_context(tc.tile_pool(name="ps_s", bufs=1, space="PSUM"))
            drampool = ctx.enter_context(tc.tile_pool(name="dram", bufs=1, space="DRAM"))
            # ---- constants ----
            woff_sb = consts.tile([128, 6, 512], F32)
            nc.sync.dma_start(out=woff_sb, in_=woff[:].rearrange("(t p) n -> p t n", p=128))
            wd_sb = consts.tile([128, 6, O], F32)
            nc.sync.dma_start(out=wd_sb, in_=wdm[:].rearrange("(t p) n -> p t n", p=128))
            ident_sb = consts.tile([PCH, PCH], F32)
            nc.sync.dma_start(out=ident_sb, in_=ident[:])
            ones_sb = consts.tile([PCH, 1], F32)
            nc.vector.memset(ones_sb, 1.0)
            offb_sb = consts.tile([PCH, 512], F32)
            nc.sync.dma_start(out=offb_sb, in_=_mkap(offb[:], 0, [[0, PCH], [1, 512]]))
            sums_sb = consts.tile([1, 1536], F32)
            nc.vector.memset(sums_sb, 0.0)
            # per-partition scalar constants for activation biases
            cbias = {}
            for s in (-2.0, -1.0, 0.0, 1.0, 2.0, EPS):
                cb = consts.tile([128, 1], F32, name=f"cb_{s}")
                nc.vector.memset(cb, float(s))
                cbias[s] = cb

            warm = consts.tile([128, 1], F32, name="warm")
            nc.scalar.activation(warm, cbias[0.0], AF.Erf, bias=cbias[0.0], scale=1.0)

            ystash = []
            # ================= phase A =================
            for ho in range(NCHUNK):
                # window load: partitions are (b, wo) b-major; one DMA per chunk
                wt = wpool.tile([PCH, NWIN], F32, name="wt")
                nc.sync.dma_start(out=wt, in_=xwin[ho])

                # patch in flat order (c, ki, kj), materialized contiguously
                patch = ptpool.tile([PCH, J], F32, name="patch")
                isrc = bass.AP(
                    tensor=wt.tensor, offset=wt.offset + PAD * WROW + PAD * C,
                    ap=[list(wt.ap[0]), [1, C], [WROW, 16], [C, 16]],
                )
                nc.vector.tensor_copy(
                    patch.rearrange("p (c ki kj) -> p c ki kj", c=C, ki=16), isrc
                )
                # patchT tiles via PE transpose of contiguous 128-slices
                ptT = ptpool.tile([128, 6, PCH], F32, name="ptT")
                for t in range(6):
                    tp = ps_t.tile([128, PCH], F32, name="tp")
                    nc.tensor.transpose(tp, patch[:, bass.ts(t, 128)], ident_sb)
                    nc.scalar.copy(out=ptT[:, t, :], in_=tp)

                # offsets matmul: out [112, 512]
                offp = ps_off.tile([PCH, 512], F32, name="offp")
                for t in range(6):
                    nc.tensor.matmul(
                        offp, lhsT=ptT[:, t, :],
                        rhs=woff_sb[:, t, :],
                        start=(t == 0), stop=(t == 5),
                    )
                dyx = dpool.tile([PCH, 512], F32, name="dyx")
                nc.vector.tensor_add(dyx, offp, offb_sb)

                # hats: lam[:, i, :] i in 0..4 -> y taps, 5..9 -> x taps
                lam = hpool.tile([PCH, 10, 256], F32, name="lam")
                for i, s in enumerate((-2, -1, 0, 1, 2)):
                    aby = hpool.tile([PCH, 256], F32, name="aby")
                    nc.scalar.activation(aby, dyx[:, 0:256], AF.Abs, bias=cbias[float(-s)][:PCH], scale=1.0)
                    nc.scalar.activation(lam[:, i, :], aby, AF.Relu, bias=cbias[1.0][:PCH], scale=-1.0)
                    abx = hpool.tile([PCH, 256], F32, name="abx")
                    nc.scalar.activation(abx, dyx[:, 256:512], AF.Abs, bias=cbias[float(-s)][:PCH], scale=1.0)
                    nc.scalar.activation(lam[:, 5 + i, :], abx, AF.Relu, bias=cbias[1.0][:PCH], scale=-1.0)

                # tap MAC: acc[p, ki, kj, c] += m[p,ki,kj] * win[p, ki+2+sy, kj+2+sx, c]
                # Two independent accumulator chains so VectorE and GpSimd
                # stream taps concurrently (GpSimd ~2x slower per op -> 8/25).
                acc = apool.tile([PCH, 768], F32, name="acc")
                accv = acc.rearrange("p (ki kj c) -> p ki kj c", ki=16, kj=16)
                accp = apool.tile([PCH, 768], F32, name="accp")
                accpv = accp.rearrange("p (ki kj c) -> p ki kj c", ki=16, kj=16)
                first_v = True
                first_p = True
                tapi = 0
                for iy in range(5):
                    sy = iy - 2
                    for ix in range(5):
                        sx = ix - 2
                        on_pool = tapi < 8
                        tapi += 1
                        eng = nc.gpsimd if on_pool else nc.vector
                        m = mpool.tile([PCH, 256], F32,
                                       name="mp" if on_pool else "m")
                        eng.tensor_mul(m, lam[:, iy, :], lam[:, 5 + ix, :])
                        mB = (
                            m.rearrange("p (ki kj) -> p ki kj", ki=16)
                            .unsqueeze(-1).broadcast_to([PCH, 16, 16, C])
                        )
                        xoff = (PAD + sy) * WROW + (PAD + sx) * C
                        xs = bass.AP(
                            tensor=wt.tensor, offset=wt.offset + xoff,
                            ap=[list(wt.ap[0]), [WROW, 16], [C, 16], [1, C]],
                        )
                        if on_pool:
                            if first_p:
                                nc.gpsimd.tensor_mul(accpv, xs, mB)
                                first_p = False
                            else:
                                tmp = tpool.tile([PCH, 768], F32, name="tmpp")
                                tv = tmp.rearrange("p (ki kj c) -> p ki kj c", ki=16, kj=16)
                                nc.gpsimd.tensor_mul(tv, xs, mB)
                                nc.gpsimd.tensor_add(accp, accp, tmp)
                        else:
                            if first_v:
                                nc.vector.tensor_mul(accv, xs, mB)
                                first_v = False
                            else:
                                tmp = tpool.tile([PCH, 768], F32, name="tmp")
                                tv = tmp.rearrange("p (ki kj c) -> p ki kj c", ki=16, kj=16)
                                nc.vector.tensor_mul(tv, xs, mB)
                                nc.vector.tensor_add(acc, acc, tmp)
                nc.vector.tensor_add(acc, acc, accp)

                # sampledT via PE transposes
                sT = stpool.tile([128, 6, PCH], F32, name="sT")
                for t in range(6):
                    tp2 = ps_t.tile([128, PCH], F32, name="tp2")
                    nc.tensor.transpose(tp2, acc[:, bass.ts(t, 128)], ident_sb)
                    nc.scalar.copy(out=sT[:, t, :], in_=tp2)

                # main matmul: y [112, 768]
                y = ypool.tile([PCH, O], F32, name="y")
                for half in range(2):
                    yp = ps_y.tile([PCH, 384], F32, name="yp")
                    for t in range(6):
                        nc.tensor.matmul(
                            yp, lhsT=sT[:, t, :],
                            rhs=wd_sb[:, t, bass.ts(half, 384)],
                            start=(t == 0), stop=(t == 5),
                        )
                    nc.scalar.copy(out=y[:, bass.ts(half, 384)], in_=yp)
                ystash.append(y)

                # BN partial sums
                ysq = sqpool.tile([PCH, O], F32, name="ysq")
                nc.scalar.activation(ysq, y, AF.Square, bias=cbias[0.0][:PCH], scale=1.0)
                for seg in range(4):
                    srcseg = (y if seg < 2 else ysq)[:, bass.ts(seg % 2, 384)]
                    sp = ps_s.tile([1, 384], F32, name="sp")
                    nc.tensor.matmul(sp, lhsT=ones_sb, rhs=srcseg,
                                     start=True, stop=True)
                    nc.vector.tensor_add(
                        sums_sb[:, bass.ts(seg, 384)], sums_sb[:, bass.ts(seg, 384)], sp
                    )

            # ================= phase B: global BN stats =================
            cc_in = drampool.tile([1, 1536], F32, name="cc_in")
            cc_out = drampool.tile([1, 1536], F32, name="cc_out", addr_space="Shared")
            nc.sync.dma_start(out=cc_in, in_=sums_sb)
            nc.gpsimd.collective_compute(
                "AllReduce", mybir.AluOpType.add,
                replica_groups=[list(range(n_cores))],
                ins=[cc_in.opt()], outs=[cc_out.opt()],
            )
            gsums = fpool.tile([128, 1536], F32)
            nc.sync.dma_start(out=gsums, in_=_mkap(cc_out, cc_out.offset, [[0, 128], [1, 1536]]))
            asc = fpool.tile([128, O], F32, name="asc")
            bsh = fpool.tile([128, O], F32, name="bsh")

            mean = fpool.tile([128, O], F32, name="ftmp", tag="ftmp", bufs=3)
            nc.scalar.mul(mean, gsums[:, 0:768], 1.0 / NTOT)
            var = fpool.tile([128, O], F32, name="ftmp2", tag="ftmp", bufs=3)
            nc.vector.tensor_mul(var, mean, mean)
            # var = E[y^2]/N - mean^2  (in place on var)
            nc.scalar.mul(gsums[:, 768:1536], gsums[:, 768:1536], 1.0 / NTOT)
            nc.vector.tensor_sub(var, gsums[:, 768:1536], var)
            # rstd = rsqrt(var + eps) via bit-trick + 3 Newton steps (DVE only)
            vpe = fpool.tile([128, O], F32, name="ftmp3", tag="ftmp", bufs=3)
            nc.vector.tensor_scalar_add(vpe, var, EPS)
            rstd = fpool.tile([128, O], F32, name="ftmp4", tag="ftmp", bufs=3)
            half_i = fpool.tile([128, O], mybir.dt.int32, name="half_i", tag="ftmpn", bufs=2)
            nc.vector.tensor_scalar(
                half_i, vpe.bitcast(mybir.dt.int32), 1, None,
                mybir.AluOpType.arith_shift_right,
            )
            nc.vector.tensor_scalar(
                half_i, half_i, -1, None, mybir.AluOpType.mult
            )
            nc.vector.tensor_scalar_add(
                rstd.bitcast(mybir.dt.int32), half_i, 0x5F3759DF
            )
            nt = fpool.tile([128, O], F32, name="ftmpn", tag="ftmpn", bufs=2)
            for _ in range(3):
                nc.vector.tensor_mul(nt, vpe, rstd)
                nc.vector.tensor_mul(nt, nt, rstd)
                nc.vector.tensor_scalar(
                    nt, nt, -0.5, 1.5, mybir.AluOpType.mult, mybir.AluOpType.add
                )
                nc.vector.tensor_mul(rstd, rstd, nt)
            # asc = gamma * rstd ; bsh = beta - mean * asc
            gam = fpool.tile([128, O], F32, name="ftmp5", tag="ftmp", bufs=3)
            nc.sync.dma_start(out=gam, in_=_mkap(bng[:], 0, [[0, 128], [1, O]]))
            nc.vector.tensor_mul(asc, gam, rstd)
            bet = fpool.tile([128, O], F32, name="ftmp6", tag="ftmp", bufs=3)
            nc.sync.dma_start(out=bet, in_=_mkap(bnb[:], 0, [[0, 128], [1, O]]))
            nc.vector.tensor_mul(bsh, mean, asc)
            nc.vector.tensor_sub(bsh, bet, bsh)

            # ================= phase C: normalize + GELU + store =================
            for ho in range(NCHUNK):
                y = ystash[ho]
                yn = cpool.tile([PCH, O], F32, name="yn")
                nc.vector.tensor_mul(yn, y, asc[:PCH, :])
                nc.vector.tensor_add(yn, yn, bsh[:PCH, :])
                g = cpool.tile([PCH, O], F32, name="g")
                nc.scalar.activation(g, yn, AF.Erf, bias=cbias[0.0][:PCH],
                                     scale=0.7071067811865476)
                nc.vector.tensor_scalar_add(g, g, 1.0)
                nc.vector.tensor_scalar_mul(g, g, 0.5)
                nc.vector.tensor_mul(g, g, yn)
                for bb in range(BL):
                    nc.sync.dma_start(
                        out=outd_b[bb, ho * WO:(ho + 1) * WO, :],
                        in_=g[bb * WO:(bb + 1) * WO, :],
                    )

    nc.compile()
    return nc


def _host_prep(x, offset_w, offset_b, dconv_w):
    xt = np.transpose(np.asarray(x, np.float32), (0, 2, 3, 1))
    xpad = np.zeros((B, HP, HP, C), np.float32)
    xpad[:, PAD:PAD + H, PAD:PAD + W, :] = xt
    # windows with halo: [B, ho, wo, 20, 20, 3] -> per-chunk layout
    sb, sy, sx, sc = xpad.strides
    win = np.lib.stride_tricks.as_strided(
        xpad, shape=(B, HO, WO, WIN, WIN, C),
        strides=(sb, 16 * sy, 16 * sx, sy, sx, sc),
    )
    # [ho, b, wo, win] per full batch
    xwin = np.ascontiguousarray(win.transpose(1, 0, 2, 3, 4, 5)).reshape(
        HO, B, WO, NWIN
    )
    woff = np.asarray(offset_w, np.float32).transpose(1, 2, 3, 0).reshape(J, 512)
    perm = np.r_[np.arange(0, 512, 2), np.arange(1, 512, 2)]
    woff = np.ascontiguousarray(woff[:, perm])
    offbp = np.ascontiguousarray(np.asarray(offset_b, np.float32)[perm])
    wd = np.ascontiguousarray(
        np.asarray(dconv_w, np.float32).transpose(2, 3, 1, 0).reshape(J, O)
    )
    return xwin, woff, offbp, wd


def kernel(x, offset_w, offset_b, dconv_w, bn_gamma, bn_beta):
    if "nc" not in _CACHE:
        _CACHE["nc"] = _build()
    nc = _CACHE["nc"]

    xwin, woff, offbp, wd = _host_prep(x, offset_w, offset_b, dconv_w)
    ident = np.eye(PCH, dtype=np.float32)
    bng = np.asarray(bn_gamma, np.float32)
    bnb = np.asarray(bn_beta, np.float32)

    in_maps = []
    for c in range(NCORES):
        in_maps.append({
            "xwin": np.ascontiguousarray(
                xwin[:, c * BL:(c + 1) * BL].reshape(NCHUNK, PCH, NWIN)),
            "woff": woff, "wdm": wd, "offb": offbp,
            "bng": bng, "bnb": bnb, "ident": ident,
        })
    res = run_bass_kernel_spmd(nc, in_maps, list(range(NCORES)))
    outs = [res.results[c]["out"] for c in range(NCORES)]
    return np.concatenate(outs, axis=0).astype(np.float32)


if __name__ == "__main__":
    # smoke: build only
    _build()
    print("build ok")
